# revision 5
# baseline (speedup 1.0000x reference)
"""Data-dependent RBF kernel for Trainium2, data-parallel over batch B=8.

Per core b:
  sigma[n]   = 0.1 + 9.9*sigmoid(MLP(emb[n]))           (tiny MLP, PE matmuls)
  out[n, m]  = exp(-((z0[m]-mu0[n])^2 + (z1[m]-mu1[n])^2) / (2 sigma[n]^2))

The distance expansion is computed with a single K=4 matmul per tile:
  -d2[n, m] = sum_k lhsT[k, n] * rhs[k, m]
  lhsT rows (aug mu): [2*mu0, 2*mu1, r_mu, 1]
  rhs  rows (aug z) : [z0,    z1,   -1,   -r_z]
followed by one ACT Exp with per-partition scale 1/(2 sigma^2).
"""

import math

import numpy as np

_B, _N, _M, _P, _E, _H, _H2 = 8, 1024, 2048, 2, 256, 32, 16
_NT = _N // 128  # 8 row tiles per core
_MT = _M // 128  # 16 z tiles

_CACHE = {}
LAST_RESULTS = None


def _install_drain_patch():
    """walrus in this container allows at most 2 sync-wait commands per
    instruction, but TileContext's final drain aggregates a wait per live
    Tile semaphore onto one Drain. Emit one Drain per wait instead."""
    import concourse.tile as _tile
    from concourse.vector_clock import ScopedClock
    from concourse import mybir as _mybir

    if getattr(_tile.TileContext, "_drain_waits_split", False):
        return

    def _split_drain_and_barrier(self, tick_clock, wait_clock):
        nc = self.nc
        probe = _mybir.InstDrain(name="probe-drain-waits")
        probe.engine = _mybir.EngineType.SP
        wait_clock.add_sem_waits(probe, ScopedClock({None: tick_clock.global_clock}))
        si = probe.sync_info
        waits = list(si.on_wait) if si is not None else []

        assert self.sems is not None
        by_name = {h.name: h for h in self.sems.allocated().values()}

        if not waits:
            nc.sync.drain()
        for w in waits:
            nc.sync.drain().wait_op(by_name[w.ant_name], w.wait_value, "sem-ge")

        nc.all_engine_barrier()
        popped = nc._tile_sem_poison_stack.pop()
        assert popped is self._sem_poison
        nc.clear_and_free_semaphores(list(self.sems.allocated().values()))
        nc.all_engine_barrier()

    _tile.TileContext._drain_and_barrier = _split_drain_and_barrier
    _tile.TileContext._drain_waits_split = True


def _install_wait_split_patch():
    """walrus in this container rejects instructions carrying more than 2
    sync-wait commands. Tile's sem assignment can attach 3+ waits to one
    instruction, so post-process the serialized BIR: excess waits move onto
    EventSemaphore instructions inserted just before the instruction on the
    same engine (engines execute in program order, so this is equivalent)."""
    import orjson
    import concourse.bass as bass

    if getattr(bass.Bass, "_wait_split_patched", False):
        return
    orig = bass.Bass.to_json_bytes
    MAXW = 1

    def to_json_bytes(self):
        j = orjson.loads(orig(self))
        cnt = 0
        for f in j.get("functions", []):
            for blk in f.get("blocks", []):
                insts = blk.get("instructions", [])
                out = []
                changed = False
                for inst in insts:
                    si = inst.get("sync_info")
                    waits = (si or {}).get("on_wait") or []
                    if len(waits) > MAXW:
                        changed = True
                        extra, keep = waits[:-MAXW], waits[-MAXW:]
                        for k in range(0, len(extra), MAXW):
                            cnt += 1
                            out.append(
                                {
                                    "debug": inst.get("debug"),
                                    "engine": inst["engine"],
                                    "ins": [],
                                    "outs": [],
                                    "name": f"waitsplit-{cnt}",
                                    "opcode": "EventSemaphore",
                                    "sync_info": {
                                        "on_update": [],
                                        "on_wait": extra[k : k + MAXW],
                                    },
                                }
                            )
                        si["on_wait"] = keep
                    out.append(inst)
                if changed:
                    blk["instructions"] = out
        return orjson.dumps(j)

    bass.Bass.to_json_bytes = to_json_bytes
    bass.Bass._wait_split_patched = True


def _build_program():
    import concourse.bass as bass
    import concourse.tile as tile
    from concourse import mybir
    from concourse.masks import make_identity

    f32 = mybir.dt.float32
    FT = mybir.ActivationFunctionType
    AX = mybir.AxisListType

    nc = bass.Bass()

    z_d = nc.dram_tensor("z", [_M, _P], f32, kind="ExternalInput")
    mu_d = nc.dram_tensor("mu", [_N, _P], f32, kind="ExternalInput")
    emb_d = nc.dram_tensor("embeddings", [_N, _E], f32, kind="ExternalInput")
    w1_d = nc.dram_tensor("w1", [_E, _H], f32, kind="ExternalInput")
    b1_d = nc.dram_tensor("b1", [_H], f32, kind="ExternalInput")
    w2_d = nc.dram_tensor("w2", [_H, _H2], f32, kind="ExternalInput")
    b2_d = nc.dram_tensor("b2", [_H2], f32, kind="ExternalInput")
    w3_d = nc.dram_tensor("w3", [_H2, 1], f32, kind="ExternalInput")
    b3_d = nc.dram_tensor("b3", [1], f32, kind="ExternalInput")
    out_d = nc.dram_tensor("out", [_N, _M], f32, kind="ExternalOutput")

    with tile.TileContext(nc) as tc:
        with (
            tc.tile_pool(name="singles", bufs=1) as singles,
            tc.tile_pool(name="psmall", bufs=2, space="PSUM") as psmall,
            tc.tile_pool(name="pmain", bufs=3, space="PSUM") as pmain,
            tc.tile_pool(name="outp", bufs=3) as outp,
        ):
            ident = singles.tile([128, 128], f32)
            make_identity(nc, ident)

            # ---------------- weights / biases ----------------
            w1_sb = singles.tile([128, 2, _H], f32)
            nc.sync.dma_start(
                out=w1_sb, in_=w1_d[:, :].rearrange("(k p) h -> p k h", p=128)
            )
            b1_sb = singles.tile([_H, 1], f32)
            nc.sync.dma_start(out=b1_sb, in_=b1_d[:].rearrange("(h o) -> h o", o=1))
            w2_sb = singles.tile([_H, _H2], f32)
            nc.sync.dma_start(out=w2_sb, in_=w2_d[:, :])
            b2_sb = singles.tile([_H2, 1], f32)
            nc.sync.dma_start(out=b2_sb, in_=b2_d[:].rearrange("(h o) -> h o", o=1))
            w3_sb = singles.tile([_H2, 1], f32)
            nc.sync.dma_start(out=w3_sb, in_=w3_d[:, :])
            b3_sb = singles.tile([128, 1], f32)
            nc.sync.dma_start(out=b3_sb, in_=b3_d[:].to_broadcast((128, 1)))

            # ---------------- embeddings load + transpose ----------------
            emb_all = singles.tile([128, _NT, _E], f32)
            for t in range(_NT):
                nc.sync.dma_start(
                    out=emb_all[:, t, :], in_=emb_d[t * 128 : (t + 1) * 128, :]
                )
            embT = singles.tile([128, 2, _N], f32)
            for e in range(2):
                for g in range(_NT // 4):
                    ps = psmall.tile([128, 512], f32, tag="ps")
                    for i in range(4):
                        t = g * 4 + i
                        nc.tensor.transpose(
                            ps[:, i * 128 : (i + 1) * 128],
                            emb_all[:, t, e * 128 : (e + 1) * 128],
                            ident,
                        )
                    nc.vector.tensor_copy(
                        out=embT[:, e, g * 512 : (g + 1) * 512], in_=ps
                    )

            # ---------------- sigma MLP ----------------
            h1 = singles.tile([_H, _N], f32)
            for j in range(2):
                ph = psmall.tile([_H, 512], f32, tag="ps")
                nc.tensor.matmul(
                    ph,
                    w1_sb[:, 0, :],
                    embT[:, 0, j * 512 : (j + 1) * 512],
                    start=True,
                    stop=False,
                )
                nc.tensor.matmul(
                    ph,
                    w1_sb[:, 1, :],
                    embT[:, 1, j * 512 : (j + 1) * 512],
                    start=False,
                    stop=True,
                )
                nc.scalar.activation(
                    out=h1[:, j * 512 : (j + 1) * 512],
                    in_=ph,
                    func=FT.Gelu,
                    bias=b1_sb,
                    scale=1.0,
                )
            h2 = singles.tile([_H2, _N], f32)
            for j in range(2):
                ph2 = psmall.tile([_H2, 512], f32, tag="ps")
                nc.tensor.matmul(
                    ph2, w2_sb, h1[:, j * 512 : (j + 1) * 512], start=True, stop=True
                )
                nc.scalar.activation(
                    out=h2[:, j * 512 : (j + 1) * 512],
                    in_=ph2,
                    func=FT.Gelu,
                    bias=b2_sb,
                    scale=1.0,
                )
            ps_s = psmall.tile([128, _NT], f32, tag="ps")
            for t in range(_NT):
                nc.tensor.matmul(
                    ps_s[:, t : t + 1],
                    h2[:, t * 128 : (t + 1) * 128],
                    w3_sb,
                    start=True,
                    stop=True,
                )
            sig = singles.tile([128, _NT], f32)
            nc.scalar.activation(
                out=sig, in_=ps_s, func=FT.Sigmoid, bias=b3_sb, scale=1.0
            )
            # 2*sigma^2 = (sqrt(2)*(0.1 + 9.9*s))^2
            sq_bias = singles.tile([128, 1], f32)
            nc.vector.memset(sq_bias, 0.1 * math.sqrt(2.0))
            two_s2 = singles.tile([128, _NT], f32)
            nc.scalar.activation(
                out=two_s2,
                in_=sig,
                func=FT.Square,
                scale=9.9 * math.sqrt(2.0),
                bias=sq_bias,
            )
            inv_sb = singles.tile([128, _NT], f32)
            nc.vector.reciprocal(out=inv_sb, in_=two_s2)

            # ---------------- z side: rhs rows [z0, z1, -1, -r_z] ----------------
            z_all = singles.tile([128, _MT, _P], f32)
            for t in range(_MT):
                nc.sync.dma_start(
                    out=z_all[:, t, :], in_=z_d[t * 128 : (t + 1) * 128, :]
                )
            zsq = singles.tile([128, _MT, _P], f32)
            nc.vector.tensor_mul(out=zsq, in0=z_all, in1=z_all)
            rz = singles.tile([128, _MT, 1], f32)
            nc.vector.reduce_sum(out=rz, in_=zsq, axis=AX.X)
            pre_z = singles.tile([128, _MT, 4], f32)
            nc.vector.memset(pre_z, -1.0)
            nc.vector.tensor_copy(out=pre_z[:, :, 0:2], in_=z_all)
            nc.vector.tensor_scalar_mul(out=pre_z[:, :, 3:4], in0=rz, scalar1=-1.0)
            rhs_sb = singles.tile([4, _MT, 128], f32)
            for g in range(_MT // 4):
                ps = psmall.tile([4, 512], f32, tag="ps")
                for i in range(4):
                    t = g * 4 + i
                    nc.tensor.transpose(
                        ps[:, i * 128 : (i + 1) * 128], pre_z[:, t, :], ident
                    )
                nc.vector.tensor_copy(out=rhs_sb[:, g * 4 : (g + 1) * 4, :], in_=ps)

            # ---------------- mu side: lhsT rows [2mu0, 2mu1, r_mu, 1] ----------------
            mu_all = singles.tile([128, _NT, _P], f32)
            for t in range(_NT):
                nc.sync.dma_start(
                    out=mu_all[:, t, :], in_=mu_d[t * 128 : (t + 1) * 128, :]
                )
            musq = singles.tile([128, _NT, _P], f32)
            nc.vector.tensor_mul(out=musq, in0=mu_all, in1=mu_all)
            pre_aug = singles.tile([128, _NT, 4], f32)
            nc.vector.memset(pre_aug, 1.0)
            nc.vector.tensor_scalar_mul(out=pre_aug[:, :, 0:2], in0=mu_all, scalar1=2.0)
            nc.vector.reduce_sum(out=pre_aug[:, :, 2:3], in_=musq, axis=AX.X)
            aug_sb = singles.tile([4, _NT, 128], f32)
            for g in range(_NT // 4):
                ps = psmall.tile([4, 512], f32, tag="ps")
                for i in range(4):
                    t = g * 4 + i
                    nc.tensor.transpose(
                        ps[:, i * 128 : (i + 1) * 128], pre_aug[:, t, :], ident
                    )
                nc.vector.tensor_copy(out=aug_sb[:, g * 4 : (g + 1) * 4, :], in_=ps)

            # ---------------- main: -d2 matmul + Exp + store ----------------
            for t in range(_NT):
                ot = outp.tile([128, _M], f32, tag="out")
                for jh in range(2):
                    pd = pmain.tile([128, 1024], f32, tag="pd")
                    for q in range(2):
                        tb = (jh * 1024 + q * 512) // 128
                        nc.tensor.matmul(
                            pd[:, q * 512 : (q + 1) * 512],
                            aug_sb[:, t, :],
                            rhs_sb[:, tb : tb + 4, :],
                            start=True,
                            stop=True,
                        )
                    nc.scalar.activation(
                        out=ot[:, jh * 1024 : (jh + 1) * 1024],
                        in_=pd,
                        func=FT.Exp,
                        scale=inv_sb[:, t : t + 1],
                    )
                nc.sync.dma_start(out=out_d[t * 128 : (t + 1) * 128, :], in_=ot)

    return nc


def kernel(z, mu, embeddings, w1, b1, w2, b2, w3, b3):
    global LAST_RESULTS
    from concourse.bass_utils import run_bass_kernel_spmd

    _install_drain_patch()
    _install_wait_split_patch()
    if "nc" not in _CACHE:
        _CACHE["nc"] = _build_program()
    nc = _CACHE["nc"]

    f = lambda a: np.ascontiguousarray(a, dtype=np.float32)
    in_maps = [
        {
            "z": f(z),
            "mu": f(mu[c]),
            "embeddings": f(embeddings[c]),
            "w1": f(w1),
            "b1": f(b1),
            "w2": f(w2),
            "b2": f(b2),
            "w3": f(w3.reshape(_H2, 1)),
            "b3": f(b3.reshape(1)),
        }
        for c in range(_B)
    ]
    res = run_bass_kernel_spmd(nc, in_maps, list(range(_B)))
    LAST_RESULTS = res
    return np.stack([res.results[c]["out"] for c in range(_B)], axis=0)


# revision 7
# speedup vs baseline: 1.1127x; 1.1127x over previous
"""Data-dependent RBF kernel for Trainium2, data-parallel over batch B=8.

Per core b:
  sigma[n]   = 0.1 + 9.9*sigmoid(MLP(emb[n]))           (tiny MLP)
  out[n, m]  = exp(-((z0[m]-mu0[n])^2 + (z1[m]-mu1[n])^2) / (2 sigma[n]^2))

All matmuls run in bf16 with two-term (hi/lo) operand splits and hi*lo
cross products so the fp32-accumulated result is accurate to ~1e-5 while
running at full bf16 PE rate (fp32 matmuls lower to the 2-pass LOW_HIGH
mode, ~5x slower, and draw enough power to trip the 50% PE throttle).

The distance expansion is one K=11 bf16 matmul per [128n x 512m] tile:
  psum[n, m] = 2*mu.z - r_z   (expansion rows below)
  out        = Exp(inv[n] * psum + (-inv[n]*r_mu[n]))    (one ACT op,
               per-partition scale/bias; inv = 1/(2 sigma^2), r_mu exact
               in fp32 via the bias so it never enters the bf16 matmul)
"""

import math

import numpy as np

_B, _N, _M, _P, _E, _H, _H2 = 8, 1024, 2048, 2, 256, 32, 16
_NT = _N // 128  # 8 row tiles per core
_MT = _M // 128  # 16 z tiles
_KR = 11  # expansion rows

_CACHE = {}
LAST_RESULTS = None


def _install_drain_patch():
    """walrus in this container allows at most 2 sync-wait commands per
    instruction, but TileContext's final drain aggregates a wait per live
    Tile semaphore onto one Drain. Emit one Drain per wait instead."""
    import concourse.tile as _tile
    from concourse.vector_clock import ScopedClock
    from concourse import mybir as _mybir

    if getattr(_tile.TileContext, "_drain_waits_split", False):
        return

    def _split_drain_and_barrier(self, tick_clock, wait_clock):
        nc = self.nc
        probe = _mybir.InstDrain(name="probe-drain-waits")
        probe.engine = _mybir.EngineType.SP
        wait_clock.add_sem_waits(probe, ScopedClock({None: tick_clock.global_clock}))
        si = probe.sync_info
        waits = list(si.on_wait) if si is not None else []

        assert self.sems is not None
        by_name = {h.name: h for h in self.sems.allocated().values()}

        if not waits:
            nc.sync.drain()
        for w in waits:
            nc.sync.drain().wait_op(by_name[w.ant_name], w.wait_value, "sem-ge")

        nc.all_engine_barrier()
        popped = nc._tile_sem_poison_stack.pop()
        assert popped is self._sem_poison
        nc.clear_and_free_semaphores(list(self.sems.allocated().values()))
        nc.all_engine_barrier()

    _tile.TileContext._drain_and_barrier = _split_drain_and_barrier
    _tile.TileContext._drain_waits_split = True


def _install_wait_split_patch():
    """walrus in this container rejects instructions carrying more than 2
    sync-wait commands (and matmuls more than ~1). Tile's sem assignment can
    attach several waits to one instruction, so post-process the serialized
    BIR: excess waits move onto EventSemaphore instructions inserted just
    before the instruction on the same engine (engines execute in program
    order, so this is equivalent)."""
    import orjson
    import concourse.bass as bass

    if getattr(bass.Bass, "_wait_split_patched", False):
        return
    orig = bass.Bass.to_json_bytes
    MAXW = 1

    def to_json_bytes(self):
        j = orjson.loads(orig(self))
        cnt = 0
        for f in j.get("functions", []):
            for blk in f.get("blocks", []):
                insts = blk.get("instructions", [])
                out = []
                changed = False
                for inst in insts:
                    si = inst.get("sync_info")
                    waits = (si or {}).get("on_wait") or []
                    if len(waits) > MAXW:
                        changed = True
                        extra, keep = waits[:-MAXW], waits[-MAXW:]
                        for k in range(0, len(extra), MAXW):
                            cnt += 1
                            out.append(
                                {
                                    "debug": inst.get("debug"),
                                    "engine": inst["engine"],
                                    "ins": [],
                                    "outs": [],
                                    "name": f"waitsplit-{cnt}",
                                    "opcode": "EventSemaphore",
                                    "sync_info": {
                                        "on_update": [],
                                        "on_wait": extra[k : k + MAXW],
                                    },
                                }
                            )
                        si["on_wait"] = keep
                    out.append(inst)
                if changed:
                    blk["instructions"] = out
        return orjson.dumps(j)

    bass.Bass.to_json_bytes = to_json_bytes
    bass.Bass._wait_split_patched = True


def _build_program():
    import concourse.bass as bass
    import concourse.tile as tile
    from concourse import mybir
    from concourse.masks import make_identity

    f32 = mybir.dt.float32
    bf16 = mybir.dt.bfloat16
    FT = mybir.ActivationFunctionType
    AX = mybir.AxisListType

    nc = bass.Bass()

    z_d = nc.dram_tensor("z", [_M, _P], f32, kind="ExternalInput")
    mu_d = nc.dram_tensor("mu", [_N, _P], f32, kind="ExternalInput")
    emb_d = nc.dram_tensor("embeddings", [_N, _E], f32, kind="ExternalInput")
    w1_d = nc.dram_tensor("w1", [_E, _H], f32, kind="ExternalInput")
    b1_d = nc.dram_tensor("b1", [_H], f32, kind="ExternalInput")
    w2_d = nc.dram_tensor("w2", [_H, _H2], f32, kind="ExternalInput")
    b2_d = nc.dram_tensor("b2", [_H2], f32, kind="ExternalInput")
    w3_d = nc.dram_tensor("w3", [_H2, 1], f32, kind="ExternalInput")
    b3_d = nc.dram_tensor("b3", [1], f32, kind="ExternalInput")
    out_d = nc.dram_tensor("out", [_N, _M], f32, kind="ExternalOutput")

    with tile.TileContext(nc) as tc:
        with (
            tc.tile_pool(name="singles", bufs=1) as singles,
            tc.tile_pool(name="psmall", bufs=2, space="PSUM") as psmall,
            tc.tile_pool(name="pmain", bufs=3, space="PSUM") as pmain,
            tc.tile_pool(name="outp", bufs=3) as outp,
        ):
            ident = singles.tile([128, 128], bf16)
            make_identity(nc, ident)
            one11 = singles.tile([1, 1], f32)
            nc.vector.memset(one11, 1.0)

            # ---------------- weights / biases (+ hi/lo splits) ----------------
            w1_f = singles.tile([128, 2, _H], f32)
            nc.sync.dma_start(
                out=w1_f, in_=w1_d[:, :].rearrange("(k p) h -> p k h", p=128)
            )
            w1_h = singles.tile([128, 2, _H], bf16)
            nc.vector.tensor_copy(out=w1_h, in_=w1_f)
            w1_lf = singles.tile([128, 2, _H], f32)
            nc.vector.tensor_sub(out=w1_lf, in0=w1_f, in1=w1_h)
            w1_l = singles.tile([128, 2, _H], bf16)
            nc.vector.tensor_copy(out=w1_l, in_=w1_lf)

            w2_f = singles.tile([_H, _H2], f32)
            nc.sync.dma_start(out=w2_f, in_=w2_d[:, :])
            w2_h = singles.tile([_H, _H2], bf16)
            nc.vector.tensor_copy(out=w2_h, in_=w2_f)
            w2_lf = singles.tile([_H, _H2], f32)
            nc.vector.tensor_sub(out=w2_lf, in0=w2_f, in1=w2_h)
            w2_l = singles.tile([_H, _H2], bf16)
            nc.vector.tensor_copy(out=w2_l, in_=w2_lf)

            w3_f = singles.tile([_H2, 1], f32)
            nc.sync.dma_start(out=w3_f, in_=w3_d[:, :])
            w3_h = singles.tile([_H2, 1], bf16)
            nc.vector.tensor_copy(out=w3_h, in_=w3_f)
            w3_lf = singles.tile([_H2, 1], f32)
            nc.vector.tensor_sub(out=w3_lf, in0=w3_f, in1=w3_h)
            w3_l = singles.tile([_H2, 1], bf16)
            nc.vector.tensor_copy(out=w3_l, in_=w3_lf)

            b1_sb = singles.tile([_H, 1], f32)
            nc.sync.dma_start(out=b1_sb, in_=b1_d[:].rearrange("(h o) -> h o", o=1))
            b2_sb = singles.tile([_H2, 1], f32)
            nc.sync.dma_start(out=b2_sb, in_=b2_d[:].rearrange("(h o) -> h o", o=1))
            b3_sb = singles.tile([128, 1], f32)
            nc.sync.dma_start(out=b3_sb, in_=b3_d[:].to_broadcast((128, 1)))

            # ---------------- embeddings: load, split, transpose ----------------
            emb_all = singles.tile([128, _NT, _E], f32)
            for t in range(_NT):
                nc.sync.dma_start(
                    out=emb_all[:, t, :], in_=emb_d[t * 128 : (t + 1) * 128, :]
                )
            emb_h = singles.tile([128, _NT, _E], bf16)
            nc.gpsimd.tensor_copy(out=emb_h, in_=emb_all)
            emb_lf = singles.tile([128, _NT, _E], f32)
            nc.gpsimd.tensor_sub(out=emb_lf, in0=emb_all, in1=emb_h)
            emb_l = singles.tile([128, _NT, _E], bf16)
            nc.gpsimd.tensor_copy(out=emb_l, in_=emb_lf)

            ehT = singles.tile([128, 2, _N], bf16)
            elT = singles.tile([128, 2, _N], bf16)
            for src, dst in ((emb_h, ehT), (emb_l, elT)):
                for e in range(2):
                    for g in range(_NT // 4):
                        ps = psmall.tile([128, 512], bf16, tag="ps")
                        for i in range(4):
                            t = g * 4 + i
                            nc.tensor.transpose(
                                ps[:, i * 128 : (i + 1) * 128],
                                src[:, t, e * 128 : (e + 1) * 128],
                                ident,
                            )
                        nc.vector.tensor_copy(
                            out=dst[:, e, g * 512 : (g + 1) * 512], in_=ps
                        )

            # ---------------- sigma MLP (split-bf16 matmuls) ----------------
            h1_f = singles.tile([_H, _N], f32)
            for j in range(2):
                ph = psmall.tile([_H, 512], f32, tag="ps")
                sl = slice(j * 512, (j + 1) * 512)
                prods = [(w1_h, ehT), (w1_l, ehT), (w1_h, elT)]
                for pi, (wsb, esb) in enumerate(prods):
                    for e in range(2):
                        nc.tensor.matmul(
                            ph,
                            wsb[:, e, :],
                            esb[:, e, sl],
                            start=(pi == 0 and e == 0),
                            stop=(pi == len(prods) - 1 and e == 1),
                        )
                nc.scalar.activation(
                    out=h1_f[:, sl], in_=ph, func=FT.Gelu, bias=b1_sb, scale=1.0
                )
            h1_h = singles.tile([_H, _N], bf16)
            nc.vector.tensor_copy(out=h1_h, in_=h1_f)
            h1_lf = singles.tile([_H, _N], f32)
            nc.vector.tensor_sub(out=h1_lf, in0=h1_f, in1=h1_h)
            h1_l = singles.tile([_H, _N], bf16)
            nc.vector.tensor_copy(out=h1_l, in_=h1_lf)

            h2_f = singles.tile([_H2, _N], f32)
            for j in range(2):
                ph2 = psmall.tile([_H2, 512], f32, tag="ps")
                sl = slice(j * 512, (j + 1) * 512)
                prods2 = [(w2_h, h1_h), (w2_l, h1_h), (w2_h, h1_l)]
                for pi, (wsb, hsb) in enumerate(prods2):
                    nc.tensor.matmul(
                        ph2,
                        wsb,
                        hsb[:, sl],
                        start=(pi == 0),
                        stop=(pi == len(prods2) - 1),
                    )
                nc.scalar.activation(
                    out=h2_f[:, sl], in_=ph2, func=FT.Gelu, bias=b2_sb, scale=1.0
                )
            h2_h = singles.tile([_H2, _N], bf16)
            nc.vector.tensor_copy(out=h2_h, in_=h2_f)
            h2_lf = singles.tile([_H2, _N], f32)
            nc.vector.tensor_sub(out=h2_lf, in0=h2_f, in1=h2_h)
            h2_l = singles.tile([_H2, _N], bf16)
            nc.vector.tensor_copy(out=h2_l, in_=h2_lf)

            # mm3 -> s_pre [1, N], then transpose slices into [128, NT]
            s_sb = singles.tile([1, _N], f32)
            for j in range(2):
                ps1 = psmall.tile([1, 512], f32, tag="ps")
                sl = slice(j * 512, (j + 1) * 512)
                prods3 = [(w3_h, h2_h), (w3_l, h2_h), (w3_h, h2_l)]
                for pi, (wsb, hsb) in enumerate(prods3):
                    nc.tensor.matmul(
                        ps1,
                        wsb,
                        hsb[:, sl],
                        start=(pi == 0),
                        stop=(pi == len(prods3) - 1),
                    )
                nc.vector.tensor_copy(out=s_sb[:, sl], in_=ps1)
            ps_s = psmall.tile([128, _NT], f32, tag="ps")
            for t in range(_NT):
                nc.tensor.transpose(
                    ps_s[:, t : t + 1], s_sb[:, t * 128 : (t + 1) * 128], one11
                )
            sig = singles.tile([128, _NT], f32)
            nc.scalar.activation(
                out=sig, in_=ps_s, func=FT.Sigmoid, bias=b3_sb, scale=1.0
            )
            # 2*sigma^2 = (sqrt(2)*(0.1 + 9.9*s))^2
            sq_bias = singles.tile([128, 1], f32)
            nc.vector.memset(sq_bias, 0.1 * math.sqrt(2.0))
            two_s2 = singles.tile([128, _NT], f32)
            nc.scalar.activation(
                out=two_s2,
                in_=sig,
                func=FT.Square,
                scale=9.9 * math.sqrt(2.0),
                bias=sq_bias,
            )
            inv_sb = singles.tile([128, _NT], f32)
            nc.vector.reciprocal(out=inv_sb, in_=two_s2)

            # ---------------- z side: moving rows ----------------
            # rows: [z0h, z0l, z0h, z1h, z1l, z1h, -r1, -r2, -r3, z0l, z1l]
            z_all = singles.tile([128, _MT, _P], f32)
            for t in range(_MT):
                nc.sync.dma_start(
                    out=z_all[:, t, :], in_=z_d[t * 128 : (t + 1) * 128, :]
                )
            pre_z = singles.tile([128, _MT, _KR], bf16)
            # hi split of z into cols 0,3 (and copies 2,5)
            nc.vector.tensor_copy(out=pre_z[:, :, 0:1], in_=z_all[:, :, 0:1])
            nc.vector.tensor_copy(out=pre_z[:, :, 3:4], in_=z_all[:, :, 1:2])
            nc.vector.tensor_copy(out=pre_z[:, :, 2:3], in_=pre_z[:, :, 0:1])
            nc.vector.tensor_copy(out=pre_z[:, :, 5:6], in_=pre_z[:, :, 3:4])
            # lo split into cols 1,4 (and copies 9,10)
            zl_f = singles.tile([128, _MT, _P], f32)
            nc.vector.tensor_sub(
                out=zl_f[:, :, 0:1], in0=z_all[:, :, 0:1], in1=pre_z[:, :, 0:1]
            )
            nc.vector.tensor_sub(
                out=zl_f[:, :, 1:2], in0=z_all[:, :, 1:2], in1=pre_z[:, :, 3:4]
            )
            nc.vector.tensor_copy(out=pre_z[:, :, 1:2], in_=zl_f[:, :, 0:1])
            nc.vector.tensor_copy(out=pre_z[:, :, 4:5], in_=zl_f[:, :, 1:2])
            nc.vector.tensor_copy(out=pre_z[:, :, 9:10], in_=pre_z[:, :, 1:2])
            nc.vector.tensor_copy(out=pre_z[:, :, 10:11], in_=pre_z[:, :, 4:5])
            # r_z 3-term split, negated, into cols 6,7,8
            zsq = singles.tile([128, _MT, _P], f32)
            nc.vector.tensor_mul(out=zsq, in0=z_all, in1=z_all)
            rz = singles.tile([128, _MT, 1], f32)
            nc.vector.reduce_sum(out=rz, in_=zsq, axis=AX.X)
            nc.vector.tensor_scalar_mul(out=pre_z[:, :, 6:7], in0=rz, scalar1=-1.0)
            rd1 = singles.tile([128, _MT, 1], f32)
            nc.vector.tensor_add(out=rd1, in0=rz, in1=pre_z[:, :, 6:7])
            nc.vector.tensor_scalar_mul(out=pre_z[:, :, 7:8], in0=rd1, scalar1=-1.0)
            rd2 = singles.tile([128, _MT, 1], f32)
            nc.vector.tensor_add(out=rd2, in0=rd1, in1=pre_z[:, :, 7:8])
            nc.vector.tensor_scalar_mul(out=pre_z[:, :, 8:9], in0=rd2, scalar1=-1.0)

            rhs_sb = singles.tile([_KR, _MT, 128], bf16)
            for g in range(_MT // 4):
                ps = psmall.tile([_KR, 512], bf16, tag="ps")
                for i in range(4):
                    t = g * 4 + i
                    nc.tensor.transpose(
                        ps[:, i * 128 : (i + 1) * 128], pre_z[:, t, :], ident
                    )
                nc.vector.tensor_copy(out=rhs_sb[:, g * 4 : (g + 1) * 4, :], in_=ps)

            # ---------------- mu side: stationary rows + r_mu bias ----------------
            # rows: [a0h, a0h, a0l, a1h, a1h, a1l, 1, 1, 1, a0l, a1l], a = 2*mu
            mu_all = singles.tile([128, _NT, _P], f32)
            for t in range(_NT):
                nc.sync.dma_start(
                    out=mu_all[:, t, :], in_=mu_d[t * 128 : (t + 1) * 128, :]
                )
            a_f = singles.tile([128, _NT, _P], f32)
            nc.vector.tensor_scalar_mul(out=a_f, in0=mu_all, scalar1=2.0)
            pre_aug = singles.tile([128, _NT, _KR], bf16)
            nc.vector.tensor_copy(out=pre_aug[:, :, 0:1], in_=a_f[:, :, 0:1])
            nc.vector.tensor_copy(out=pre_aug[:, :, 3:4], in_=a_f[:, :, 1:2])
            nc.vector.tensor_copy(out=pre_aug[:, :, 1:2], in_=pre_aug[:, :, 0:1])
            nc.vector.tensor_copy(out=pre_aug[:, :, 4:5], in_=pre_aug[:, :, 3:4])
            al_f = singles.tile([128, _NT, _P], f32)
            nc.vector.tensor_sub(
                out=al_f[:, :, 0:1], in0=a_f[:, :, 0:1], in1=pre_aug[:, :, 0:1]
            )
            nc.vector.tensor_sub(
                out=al_f[:, :, 1:2], in0=a_f[:, :, 1:2], in1=pre_aug[:, :, 3:4]
            )
            nc.vector.tensor_copy(out=pre_aug[:, :, 2:3], in_=al_f[:, :, 0:1])
            nc.vector.tensor_copy(out=pre_aug[:, :, 5:6], in_=al_f[:, :, 1:2])
            nc.vector.tensor_copy(out=pre_aug[:, :, 9:10], in_=pre_aug[:, :, 2:3])
            nc.vector.tensor_copy(out=pre_aug[:, :, 10:11], in_=pre_aug[:, :, 5:6])
            nc.vector.memset(pre_aug[:, :, 6:9], 1.0)

            aug_sb = singles.tile([_KR, _NT, 128], bf16)
            for g in range(_NT // 4):
                ps = psmall.tile([_KR, 512], bf16, tag="ps")
                for i in range(4):
                    t = g * 4 + i
                    nc.tensor.transpose(
                        ps[:, i * 128 : (i + 1) * 128], pre_aug[:, t, :], ident
                    )
                nc.vector.tensor_copy(out=aug_sb[:, g * 4 : (g + 1) * 4, :], in_=ps)

            # bias = -inv * r_mu (exact fp32; r_mu never enters the bf16 matmul)
            musq = singles.tile([128, _NT, _P], f32)
            nc.vector.tensor_mul(out=musq, in0=mu_all, in1=mu_all)
            rmu = singles.tile([128, _NT], f32)
            nc.vector.reduce_sum(
                out=rmu.rearrange("p (t o) -> p t o", o=1), in_=musq, axis=AX.X
            )
            nbias = singles.tile([128, _NT], f32)
            nc.vector.tensor_mul(out=nbias, in0=inv_sb, in1=rmu)
            nc.vector.tensor_scalar_mul(out=nbias, in0=nbias, scalar1=-1.0)

            # ---------------- main: bf16 matmul + Exp + store ----------------
            for t in range(_NT):
                ot = outp.tile([128, _M], f32, tag="out")
                for jh in range(2):
                    pd = pmain.tile([128, 1024], f32, tag="pd")
                    for q in range(2):
                        tb = (jh * 1024 + q * 512) // 128
                        nc.tensor.matmul(
                            pd[:, q * 512 : (q + 1) * 512],
                            aug_sb[:, t, :],
                            rhs_sb[:, tb : tb + 4, :],
                            start=True,
                            stop=True,
                        )
                    nc.scalar.activation(
                        out=ot[:, jh * 1024 : (jh + 1) * 1024],
                        in_=pd,
                        func=FT.Exp,
                        scale=inv_sb[:, t : t + 1],
                        bias=nbias[:, t : t + 1],
                    )
                nc.sync.dma_start(out=out_d[t * 128 : (t + 1) * 128, :], in_=ot)

    return nc


def kernel(z, mu, embeddings, w1, b1, w2, b2, w3, b3):
    global LAST_RESULTS
    from concourse.bass_utils import run_bass_kernel_spmd

    _install_drain_patch()
    _install_wait_split_patch()
    if "nc" not in _CACHE:
        _CACHE["nc"] = _build_program()
    nc = _CACHE["nc"]

    f = lambda a: np.ascontiguousarray(a, dtype=np.float32)
    in_maps = [
        {
            "z": f(z),
            "mu": f(mu[c]),
            "embeddings": f(embeddings[c]),
            "w1": f(w1),
            "b1": f(b1),
            "w2": f(w2),
            "b2": f(b2),
            "w3": f(w3.reshape(_H2, 1)),
            "b3": f(b3.reshape(1)),
        }
        for c in range(_B)
    ]
    res = run_bass_kernel_spmd(nc, in_maps, list(range(_B)))
    LAST_RESULTS = res
    return np.stack([res.results[c]["out"] for c in range(_B)], axis=0)


# revision 8
# speedup vs baseline: 1.4501x; 1.3032x over previous
"""Data-dependent RBF kernel for Trainium2, data-parallel over batch B=8.

Per core b:
  sigma[n]   = 0.1 + 9.9*sigmoid(MLP(emb[n]))           (tiny MLP)
  out[n, m]  = exp(-((z0[m]-mu0[n])^2 + (z1[m]-mu1[n])^2) / (2 sigma[n]^2))

All matmuls run in bf16 with two-term (hi/lo) operand splits and hi*lo
cross products so the fp32-accumulated result is accurate to ~1e-5 while
running at full bf16 PE rate (fp32 matmuls lower to the 2-pass LOW_HIGH
mode, ~5x slower, and draw enough power to trip the 50% PE throttle).

The distance expansion is one K=11 bf16 matmul per [128n x 512m] tile:
  psum[n, m] = 2*mu.z - r_z   (expansion rows below)
  out        = Exp(inv[n] * psum + (-inv[n]*r_mu[n]))    (one ACT op,
               per-partition scale/bias; inv = 1/(2 sigma^2), r_mu exact
               in fp32 via the bias so it never enters the bf16 matmul)
"""

import math

import numpy as np

_B, _N, _M, _P, _E, _H, _H2 = 8, 1024, 2048, 2, 256, 32, 16
_NT = _N // 128  # 8 row tiles per core
_MT = _M // 128  # 16 z tiles
_KR = 11  # expansion rows

_CACHE = {}
LAST_RESULTS = None


def _install_drain_patch():
    """walrus in this container allows at most 2 sync-wait commands per
    instruction, but TileContext's final drain aggregates a wait per live
    Tile semaphore onto one Drain. Emit one Drain per wait instead."""
    import concourse.tile as _tile
    from concourse.vector_clock import ScopedClock
    from concourse import mybir as _mybir

    if getattr(_tile.TileContext, "_drain_waits_split", False):
        return

    def _split_drain_and_barrier(self, tick_clock, wait_clock):
        nc = self.nc
        probe = _mybir.InstDrain(name="probe-drain-waits")
        probe.engine = _mybir.EngineType.SP
        wait_clock.add_sem_waits(probe, ScopedClock({None: tick_clock.global_clock}))
        si = probe.sync_info
        waits = list(si.on_wait) if si is not None else []

        assert self.sems is not None
        by_name = {h.name: h for h in self.sems.allocated().values()}

        if not waits:
            nc.sync.drain()
        for w in waits:
            nc.sync.drain().wait_op(by_name[w.ant_name], w.wait_value, "sem-ge")

        nc.all_engine_barrier()
        popped = nc._tile_sem_poison_stack.pop()
        assert popped is self._sem_poison
        nc.clear_and_free_semaphores(list(self.sems.allocated().values()))
        nc.all_engine_barrier()

    _tile.TileContext._drain_and_barrier = _split_drain_and_barrier
    _tile.TileContext._drain_waits_split = True


def _install_wait_split_patch():
    """walrus in this container rejects instructions carrying more than 2
    sync-wait commands (and matmuls more than ~1). Tile's sem assignment can
    attach several waits to one instruction, so post-process the serialized
    BIR: excess waits move onto EventSemaphore instructions inserted just
    before the instruction on the same engine (engines execute in program
    order, so this is equivalent)."""
    import orjson
    import concourse.bass as bass

    if getattr(bass.Bass, "_wait_split_patched", False):
        return
    orig = bass.Bass.to_json_bytes
    MAXW = 1

    def to_json_bytes(self):
        j = orjson.loads(orig(self))
        cnt = 0
        for f in j.get("functions", []):
            for blk in f.get("blocks", []):
                insts = blk.get("instructions", [])
                out = []
                changed = False
                for inst in insts:
                    si = inst.get("sync_info")
                    waits = (si or {}).get("on_wait") or []
                    if len(waits) > MAXW:
                        changed = True
                        extra, keep = waits[:-MAXW], waits[-MAXW:]
                        for k in range(0, len(extra), MAXW):
                            cnt += 1
                            out.append(
                                {
                                    "debug": inst.get("debug"),
                                    "engine": inst["engine"],
                                    "ins": [],
                                    "outs": [],
                                    "name": f"waitsplit-{cnt}",
                                    "opcode": "EventSemaphore",
                                    "sync_info": {
                                        "on_update": [],
                                        "on_wait": extra[k : k + MAXW],
                                    },
                                }
                            )
                        si["on_wait"] = keep
                    out.append(inst)
                if changed:
                    blk["instructions"] = out
        return orjson.dumps(j)

    bass.Bass.to_json_bytes = to_json_bytes
    bass.Bass._wait_split_patched = True


def _build_program():
    import concourse.bass as bass
    import concourse.tile as tile
    from concourse import mybir
    from concourse.masks import make_identity

    f32 = mybir.dt.float32
    bf16 = mybir.dt.bfloat16
    FT = mybir.ActivationFunctionType
    AX = mybir.AxisListType

    nc = bass.Bass()

    z_d = nc.dram_tensor("z", [_M, _P], f32, kind="ExternalInput")
    mu_d = nc.dram_tensor("mu", [_N, _P], f32, kind="ExternalInput")
    emb_d = nc.dram_tensor("embeddings", [_N, _E], f32, kind="ExternalInput")
    w1_d = nc.dram_tensor("w1", [_E, _H], f32, kind="ExternalInput")
    b1_d = nc.dram_tensor("b1", [_H], f32, kind="ExternalInput")
    w2_d = nc.dram_tensor("w2", [_H, _H2], f32, kind="ExternalInput")
    b2_d = nc.dram_tensor("b2", [_H2], f32, kind="ExternalInput")
    w3_d = nc.dram_tensor("w3", [_H2, 1], f32, kind="ExternalInput")
    b3_d = nc.dram_tensor("b3", [1], f32, kind="ExternalInput")
    out_d = nc.dram_tensor("out", [_N, _M], f32, kind="ExternalOutput")

    with tile.TileContext(nc) as tc:
        with (
            tc.tile_pool(name="singles", bufs=1) as singles,
            tc.tile_pool(name="psmall", bufs=2, space="PSUM") as psmall,
            tc.tile_pool(name="pmain", bufs=3, space="PSUM") as pmain,
            tc.tile_pool(name="outp", bufs=3) as outp,
        ):
            ident = singles.tile([128, 128], bf16)
            make_identity(nc, ident)
            one11 = singles.tile([1, 1], f32)
            nc.vector.memset(one11, 1.0)

            # ---------------- weights / biases (+ hi/lo splits) ----------------
            w1_f = singles.tile([128, 2, _H], f32)
            nc.gpsimd.dma_start(
                out=w1_f, in_=w1_d[:, :].rearrange("(k p) h -> p k h", p=128)
            )
            w1_h = singles.tile([128, 2, _H], bf16)
            nc.vector.tensor_copy(out=w1_h, in_=w1_f)
            w1_lf = singles.tile([128, 2, _H], f32)
            nc.vector.tensor_sub(out=w1_lf, in0=w1_f, in1=w1_h)
            w1_l = singles.tile([128, 2, _H], bf16)
            nc.vector.tensor_copy(out=w1_l, in_=w1_lf)

            w2_f = singles.tile([_H, _H2], f32)
            nc.gpsimd.dma_start(out=w2_f, in_=w2_d[:, :])
            w2_h = singles.tile([_H, _H2], bf16)
            nc.vector.tensor_copy(out=w2_h, in_=w2_f)
            w2_lf = singles.tile([_H, _H2], f32)
            nc.vector.tensor_sub(out=w2_lf, in0=w2_f, in1=w2_h)
            w2_l = singles.tile([_H, _H2], bf16)
            nc.vector.tensor_copy(out=w2_l, in_=w2_lf)

            w3_f = singles.tile([_H2, 1], f32)
            nc.gpsimd.dma_start(out=w3_f, in_=w3_d[:, :])
            w3_h = singles.tile([_H2, 1], bf16)
            nc.vector.tensor_copy(out=w3_h, in_=w3_f)
            w3_lf = singles.tile([_H2, 1], f32)
            nc.vector.tensor_sub(out=w3_lf, in0=w3_f, in1=w3_h)
            w3_l = singles.tile([_H2, 1], bf16)
            nc.vector.tensor_copy(out=w3_l, in_=w3_lf)

            b1_sb = singles.tile([_H, 1], f32)
            nc.gpsimd.dma_start(out=b1_sb, in_=b1_d[:].rearrange("(h o) -> h o", o=1))
            b2_sb = singles.tile([_H2, 1], f32)
            nc.gpsimd.dma_start(out=b2_sb, in_=b2_d[:].rearrange("(h o) -> h o", o=1))
            b3_sb = singles.tile([128, 1], f32)
            nc.gpsimd.dma_start(out=b3_sb, in_=b3_d[:].to_broadcast((128, 1)))

            # ---------------- embeddings: load, split, transpose ----------------
            emb_all = singles.tile([128, _NT, _E], f32)
            nc.sync.dma_start(
                out=emb_all, in_=emb_d[:, :].rearrange("(t p) e -> p t e", p=128)
            )
            emb_h = singles.tile([128, _NT, _E], bf16)
            nc.vector.tensor_copy(out=emb_h, in_=emb_all)
            emb_lf = singles.tile([128, _NT, _E], f32)
            nc.vector.tensor_sub(out=emb_lf, in0=emb_all, in1=emb_h)
            emb_l = singles.tile([128, _NT, _E], bf16)
            nc.vector.tensor_copy(out=emb_l, in_=emb_lf)

            ehT = singles.tile([128, 2, _N], bf16)
            elT = singles.tile([128, 2, _N], bf16)
            for src, dst in ((emb_h, ehT), (emb_l, elT)):
                for e in range(2):
                    for g in range(_NT // 4):
                        ps = psmall.tile([128, 512], bf16, tag="ps")
                        for i in range(4):
                            t = g * 4 + i
                            nc.tensor.transpose(
                                ps[:, i * 128 : (i + 1) * 128],
                                src[:, t, e * 128 : (e + 1) * 128],
                                ident,
                            )
                        nc.vector.tensor_copy(
                            out=dst[:, e, g * 512 : (g + 1) * 512], in_=ps
                        )

            # ---------------- sigma MLP (split-bf16 matmuls) ----------------
            h1_f = singles.tile([_H, _N], f32)
            for j in range(2):
                ph = psmall.tile([_H, 512], f32, tag="ps")
                sl = slice(j * 512, (j + 1) * 512)
                prods = [(w1_h, ehT), (w1_l, ehT), (w1_h, elT)]
                for pi, (wsb, esb) in enumerate(prods):
                    for e in range(2):
                        nc.tensor.matmul(
                            ph,
                            wsb[:, e, :],
                            esb[:, e, sl],
                            start=(pi == 0 and e == 0),
                            stop=(pi == len(prods) - 1 and e == 1),
                        )
                nc.scalar.activation(
                    out=h1_f[:, sl], in_=ph, func=FT.Gelu, bias=b1_sb, scale=1.0
                )
            h1_h = singles.tile([_H, _N], bf16)
            nc.vector.tensor_copy(out=h1_h, in_=h1_f)
            h1_lf = singles.tile([_H, _N], f32)
            nc.vector.tensor_sub(out=h1_lf, in0=h1_f, in1=h1_h)
            h1_l = singles.tile([_H, _N], bf16)
            nc.vector.tensor_copy(out=h1_l, in_=h1_lf)

            h2_f = singles.tile([_H2, _N], f32)
            for j in range(2):
                ph2 = psmall.tile([_H2, 512], f32, tag="ps")
                sl = slice(j * 512, (j + 1) * 512)
                prods2 = [(w2_h, h1_h), (w2_l, h1_h), (w2_h, h1_l)]
                for pi, (wsb, hsb) in enumerate(prods2):
                    nc.tensor.matmul(
                        ph2,
                        wsb,
                        hsb[:, sl],
                        start=(pi == 0),
                        stop=(pi == len(prods2) - 1),
                    )
                nc.scalar.activation(
                    out=h2_f[:, sl], in_=ph2, func=FT.Gelu, bias=b2_sb, scale=1.0
                )
            h2_h = singles.tile([_H2, _N], bf16)
            nc.vector.tensor_copy(out=h2_h, in_=h2_f)
            h2_lf = singles.tile([_H2, _N], f32)
            nc.vector.tensor_sub(out=h2_lf, in0=h2_f, in1=h2_h)
            h2_l = singles.tile([_H2, _N], bf16)
            nc.vector.tensor_copy(out=h2_l, in_=h2_lf)

            # mm3 -> s_pre [1, N], then transpose slices into [128, NT]
            s_sb = singles.tile([1, _N], f32)
            for j in range(2):
                ps1 = psmall.tile([1, 512], f32, tag="ps")
                sl = slice(j * 512, (j + 1) * 512)
                prods3 = [(w3_h, h2_h), (w3_l, h2_h), (w3_h, h2_l)]
                for pi, (wsb, hsb) in enumerate(prods3):
                    nc.tensor.matmul(
                        ps1,
                        wsb,
                        hsb[:, sl],
                        start=(pi == 0),
                        stop=(pi == len(prods3) - 1),
                    )
                nc.vector.tensor_copy(out=s_sb[:, sl], in_=ps1)
            ps_s = psmall.tile([128, _NT], f32, tag="ps")
            for t in range(_NT):
                nc.tensor.transpose(
                    ps_s[:, t : t + 1], s_sb[:, t * 128 : (t + 1) * 128], one11
                )
            sig = singles.tile([128, _NT], f32)
            nc.scalar.activation(
                out=sig, in_=ps_s, func=FT.Sigmoid, bias=b3_sb, scale=1.0
            )
            # 2*sigma^2 = (sqrt(2)*(0.1 + 9.9*s))^2
            sq_bias = singles.tile([128, 1], f32)
            nc.vector.memset(sq_bias, 0.1 * math.sqrt(2.0))
            two_s2 = singles.tile([128, _NT], f32)
            nc.scalar.activation(
                out=two_s2,
                in_=sig,
                func=FT.Square,
                scale=9.9 * math.sqrt(2.0),
                bias=sq_bias,
            )
            inv_sb = singles.tile([128, _NT], f32)
            nc.vector.reciprocal(out=inv_sb, in_=two_s2)

            # ---------------- z side: moving rows ----------------
            # rows: [z0h, z0l, z0h, z1h, z1l, z1h, -r1, -r2, -r3, z0l, z1l]
            z_all = singles.tile([128, _MT, _P], f32)
            nc.gpsimd.dma_start(
                out=z_all, in_=z_d[:, :].rearrange("(t p) c -> p t c", p=128)
            )
            pre_z = singles.tile([128, _MT, _KR], bf16)
            # hi split of z into cols 0,3 (and copies 2,5)
            nc.vector.tensor_copy(out=pre_z[:, :, 0:1], in_=z_all[:, :, 0:1])
            nc.vector.tensor_copy(out=pre_z[:, :, 3:4], in_=z_all[:, :, 1:2])
            nc.vector.tensor_copy(out=pre_z[:, :, 2:3], in_=pre_z[:, :, 0:1])
            nc.vector.tensor_copy(out=pre_z[:, :, 5:6], in_=pre_z[:, :, 3:4])
            # lo split into cols 1,4 (and copies 9,10)
            zl_f = singles.tile([128, _MT, _P], f32)
            nc.vector.tensor_sub(
                out=zl_f[:, :, 0:1], in0=z_all[:, :, 0:1], in1=pre_z[:, :, 0:1]
            )
            nc.vector.tensor_sub(
                out=zl_f[:, :, 1:2], in0=z_all[:, :, 1:2], in1=pre_z[:, :, 3:4]
            )
            nc.vector.tensor_copy(out=pre_z[:, :, 1:2], in_=zl_f[:, :, 0:1])
            nc.vector.tensor_copy(out=pre_z[:, :, 4:5], in_=zl_f[:, :, 1:2])
            nc.vector.tensor_copy(out=pre_z[:, :, 9:10], in_=pre_z[:, :, 1:2])
            nc.vector.tensor_copy(out=pre_z[:, :, 10:11], in_=pre_z[:, :, 4:5])
            # r_z 3-term split, negated, into cols 6,7,8
            zsq = singles.tile([128, _MT, _P], f32)
            nc.vector.tensor_mul(out=zsq, in0=z_all, in1=z_all)
            rz = singles.tile([128, _MT, 1], f32)
            nc.vector.reduce_sum(out=rz, in_=zsq, axis=AX.X)
            nc.vector.tensor_scalar_mul(out=pre_z[:, :, 6:7], in0=rz, scalar1=-1.0)
            rd1 = singles.tile([128, _MT, 1], f32)
            nc.vector.tensor_add(out=rd1, in0=rz, in1=pre_z[:, :, 6:7])
            nc.vector.tensor_scalar_mul(out=pre_z[:, :, 7:8], in0=rd1, scalar1=-1.0)
            rd2 = singles.tile([128, _MT, 1], f32)
            nc.vector.tensor_add(out=rd2, in0=rd1, in1=pre_z[:, :, 7:8])
            nc.vector.tensor_scalar_mul(out=pre_z[:, :, 8:9], in0=rd2, scalar1=-1.0)

            rhs_sb = singles.tile([_KR, _MT, 128], bf16)
            for g in range(_MT // 4):
                ps = psmall.tile([_KR, 512], bf16, tag="ps")
                for i in range(4):
                    t = g * 4 + i
                    nc.tensor.transpose(
                        ps[:, i * 128 : (i + 1) * 128], pre_z[:, t, :], ident
                    )
                nc.vector.tensor_copy(out=rhs_sb[:, g * 4 : (g + 1) * 4, :], in_=ps)

            # ---------------- mu side: stationary rows + r_mu bias ----------------
            # rows: [a0h, a0h, a0l, a1h, a1h, a1l, 1, 1, 1, a0l, a1l], a = 2*mu
            mu_all = singles.tile([128, _NT, _P], f32)
            nc.gpsimd.dma_start(
                out=mu_all, in_=mu_d[:, :].rearrange("(t p) c -> p t c", p=128)
            )
            a_f = singles.tile([128, _NT, _P], f32)
            nc.vector.tensor_scalar_mul(out=a_f, in0=mu_all, scalar1=2.0)
            pre_aug = singles.tile([128, _NT, _KR], bf16)
            nc.vector.tensor_copy(out=pre_aug[:, :, 0:1], in_=a_f[:, :, 0:1])
            nc.vector.tensor_copy(out=pre_aug[:, :, 3:4], in_=a_f[:, :, 1:2])
            nc.vector.tensor_copy(out=pre_aug[:, :, 1:2], in_=pre_aug[:, :, 0:1])
            nc.vector.tensor_copy(out=pre_aug[:, :, 4:5], in_=pre_aug[:, :, 3:4])
            al_f = singles.tile([128, _NT, _P], f32)
            nc.vector.tensor_sub(
                out=al_f[:, :, 0:1], in0=a_f[:, :, 0:1], in1=pre_aug[:, :, 0:1]
            )
            nc.vector.tensor_sub(
                out=al_f[:, :, 1:2], in0=a_f[:, :, 1:2], in1=pre_aug[:, :, 3:4]
            )
            nc.vector.tensor_copy(out=pre_aug[:, :, 2:3], in_=al_f[:, :, 0:1])
            nc.vector.tensor_copy(out=pre_aug[:, :, 5:6], in_=al_f[:, :, 1:2])
            nc.vector.tensor_copy(out=pre_aug[:, :, 9:10], in_=pre_aug[:, :, 2:3])
            nc.vector.tensor_copy(out=pre_aug[:, :, 10:11], in_=pre_aug[:, :, 5:6])
            nc.vector.memset(pre_aug[:, :, 6:9], 1.0)

            aug_sb = singles.tile([_KR, _NT, 128], bf16)
            for g in range(_NT // 4):
                ps = psmall.tile([_KR, 512], bf16, tag="ps")
                for i in range(4):
                    t = g * 4 + i
                    nc.tensor.transpose(
                        ps[:, i * 128 : (i + 1) * 128], pre_aug[:, t, :], ident
                    )
                nc.vector.tensor_copy(out=aug_sb[:, g * 4 : (g + 1) * 4, :], in_=ps)

            # bias = -inv * r_mu (exact fp32; r_mu never enters the bf16 matmul)
            musq = singles.tile([128, _NT, _P], f32)
            nc.vector.tensor_mul(out=musq, in0=mu_all, in1=mu_all)
            rmu = singles.tile([128, _NT], f32)
            nc.vector.reduce_sum(
                out=rmu.rearrange("p (t o) -> p t o", o=1), in_=musq, axis=AX.X
            )
            nbias = singles.tile([128, _NT], f32)
            nc.vector.tensor_mul(out=nbias, in0=inv_sb, in1=rmu)
            nc.vector.tensor_scalar_mul(out=nbias, in0=nbias, scalar1=-1.0)

            # ---------------- main: bf16 matmul + Exp + store ----------------
            for t in range(_NT):
                ot = outp.tile([128, _M], f32, tag="out")
                for jh in range(2):
                    pd = pmain.tile([128, 1024], f32, tag="pd")
                    for q in range(2):
                        tb = (jh * 1024 + q * 512) // 128
                        nc.tensor.matmul(
                            pd[:, q * 512 : (q + 1) * 512],
                            aug_sb[:, t, :],
                            rhs_sb[:, tb : tb + 4, :],
                            start=True,
                            stop=True,
                        )
                    nc.scalar.activation(
                        out=ot[:, jh * 1024 : (jh + 1) * 1024],
                        in_=pd,
                        func=FT.Exp,
                        scale=inv_sb[:, t : t + 1],
                        bias=nbias[:, t : t + 1],
                    )
                nc.sync.dma_start(out=out_d[t * 128 : (t + 1) * 128, :], in_=ot)

    return nc


def kernel(z, mu, embeddings, w1, b1, w2, b2, w3, b3):
    global LAST_RESULTS
    from concourse.bass_utils import run_bass_kernel_spmd

    _install_drain_patch()
    _install_wait_split_patch()
    if "nc" not in _CACHE:
        _CACHE["nc"] = _build_program()
    nc = _CACHE["nc"]

    f = lambda a: np.ascontiguousarray(a, dtype=np.float32)
    in_maps = [
        {
            "z": f(z),
            "mu": f(mu[c]),
            "embeddings": f(embeddings[c]),
            "w1": f(w1),
            "b1": f(b1),
            "w2": f(w2),
            "b2": f(b2),
            "w3": f(w3.reshape(_H2, 1)),
            "b3": f(b3.reshape(1)),
        }
        for c in range(_B)
    ]
    res = run_bass_kernel_spmd(nc, in_maps, list(range(_B)))
    LAST_RESULTS = res
    return np.stack([res.results[c]["out"] for c in range(_B)], axis=0)
